# revision 39
# baseline (speedup 1.0000x reference)
"""Trainium2 Bass kernel v2 for nn_Evaluation_78383153152424.

Sharding: 8 cores = 2 batches x 4 D-groups (8 planes each). Zero collectives.

vs v1:
  - Stage-1 as two concurrent K=64 row-tiles ((0,0)/(64,0)), stage-2 as two
    concurrent M=64 col-tiles ((0,0)/(0,64)): ~2x PE throughput on the conv.
  - Gather muls are single-term FLAT DVE ops over [128, gn*XPAD] (measured:
    flat TT runs ~2.5x faster than 4-dim-AP TT). Pad columns carry reflected
    sim values and zero weights, so junk never escapes.
  - Center term folded: narrow(s=4) and wide(s=4) share the zero shift, so
    17 mul terms instead of 18.
  - Gather reduction engine per group is tunable: DVE flat add tree or PE
    identity-accumulate matmuls (kills the DVE-only tail).
  - relu1 on ACT, relu2 on DVE by default (per-site knobs).
  - 2 groups of 4 planes; per-block ps3 (1 bank) sim evac on ACT.
"""

import os
import sys
import functools

import numpy as np

for _p in ("/opt/trn_rl_repo", "/root/.axon_site/_ro/trn_rl_repo"):
    if os.path.isdir(_p) and _p not in sys.path:
        sys.path.append(_p)

import concourse.bass as bass
import concourse.tile as tile
from concourse import bacc, mybir
from concourse.bass_utils import run_bass_kernel_spmd

F16, F32 = mybir.dt.float16, mybir.dt.float32
AF = mybir.ActivationFunctionType
OP = mybir.AluOpType

B, G, D, H, W = 2, 8, 32, 128, 160
DG = 8
NCHUNK, RPC = 16, 8
CHUNK_F = RPC * W            # 1280
BLOCKS = [(0, 512), (512, 512), (1024, 256)]
XPAD = W + 8                 # 168


def _env(name, default):
    return os.environ.get(name, default)

GROUPS = [int(c) for c in _env("K_GROUPS", "44")]
GMAX = max(GROUPS)
WARMUP_MM = int(_env("K_WARMUP", "30"))
R1_ENG = _env("K_R1", "a" * 24)      # relu1 engine per (plane*3+k): a/v
# relu2: group-0 sites on DVE (free during conv g0), group-1 sites on ACT so
# conv g1's tail never queues behind gather g0's DVE burst
R2_ENG = _env("K_R2", "v" * 12 + "a" * 12)
SE_ENG = _env("K_SE", "a" * 8)       # sim evac engine per (grp*3+k): a/v
TREE_ENG = _env("K_TREE", "vp")      # per group: v=DVE tree, p=PE matmuls
MUL_GP = int(_env("K_MULGP", "5"))   # how many mul terms per group on gpsimd
RLD_ENG = _env("K_RLD", "s")         # yvall reload issue queue: g/s

# 17 gather terms: (shift_slot, dx, wall_slice). Wide terms s=0..8 use wall
# slice s (slice 4 pre-combined with narrow slice 13); narrow terms s!=4 use
# wall slice 9+s.
TERMS = []
for _s in range(9):
    _iy, _ix = _s // 3, _s % 3
    TERMS.append((2 * _iy, (_ix - 1) * 4))        # wide: dy=4(iy-1), slot=2iy
for _s in range(9):
    if _s == 4:
        continue
    _iy, _ix = _s // 3, _s % 3
    TERMS.append((_iy + 1, (_ix - 1) * 2))        # narrow: dy=2(iy-1), slot=iy+1
NTERM = len(TERMS)  # 17
YOFF = 4


def _ap(t, extra_off, dims):
    return bass.AP(tensor=t.tensor, offset=t.offset + extra_off, ap=dims)


@functools.lru_cache(maxsize=4)
def _build(zb0, zb1, zbsim):
    assert zb0 and zb1 and zbsim, "non-zero biases not supported in v2"
    nc = bacc.Bacc("TRN2", target_bir_lowering=False, debug=False, num_devices=8)

    x_ap = nc.dram_tensor("x", [NCHUNK, G, DG * CHUNK_F], F16,
                          kind="ExternalInput").ap()
    # wts cols: [s01: 128][ta: 64][l3e: 32][l3o: 32][ident: 128]
    wts_ap = nc.dram_tensor("wts", [128, 384], F16, kind="ExternalInput").ap()
    # host-prefolded gather weights, wrep layout [row, t*GMAX*XPAD+j*XPAD+xp]
    # (wall[ws_t, r, xp-4] in xp 4..164, zero x-pads)
    wreph_ap = nc.dram_tensor("wreph", [128, NTERM * GMAX * XPAD], F16,
                              kind="ExternalInput").ap()
    out_ap = nc.dram_tensor("out", [DG, H, W], F32, kind="ExternalOutput").ap()
    # one scratch tensor per group so group g+1's stores never serialize
    # against group g's reloads via whole-tensor dependency tracking
    scr_ts = [nc.dram_tensor(f"scr{g}", [gn, H + 8, W], F16).ap()
              for g, gn in enumerate(GROUPS)]

    import contextlib
    with tile.TileContext(nc) as tc, contextlib.ExitStack() as ctx:
        wp = ctx.enter_context(tc.tile_pool(name="wp", bufs=1))
        xp = ctx.enter_context(tc.tile_pool(name="xp", bufs=2))
        hp = ctx.enter_context(tc.tile_pool(name="hp", bufs=3))
        h2p = ctx.enter_context(tc.tile_pool(name="h2p", bufs=4))
        sfp = ctx.enter_context(tc.tile_pool(name="sfp", bufs=2))
        yvp = ctx.enter_context(tc.tile_pool(name="yvp", bufs=2))
        ptp = ctx.enter_context(tc.tile_pool(name="ptp", bufs=2))
        ofp = ctx.enter_context(tc.tile_pool(name="ofp", bufs=2))
        ps1p = ctx.enter_context(tc.tile_pool(name="ps1p", bufs=2, space="PSUM"))
        ps2p = ctx.enter_context(tc.tile_pool(name="ps2p", bufs=2, space="PSUM"))
        ps3p = ctx.enter_context(tc.tile_pool(name="ps3p", bufs=2, space="PSUM"))

        wts = wp.tile([128, 384], F16)
        nc.sync.dma_start(out=wts[:], in_=wts_ap[:])
        s01 = wts[:, 0:128]
        ta = wts[:, 128:192]
        l3e = wts[:, 192:224]
        l3o = wts[:, 224:256]
        ident = wts[:, 256:384]

        # x loads in 2-plane chunks so conv starts early
        xts = {}
        plane0 = 0
        for grp, gn in enumerate(GROUPS):
            xt = xp.tile([128, gn * CHUNK_F], F16, tag="x", name=f"xt{grp}")
            for jj in range(0, gn, 2):
                nn = min(2, gn - jj)
                nc.sync.dma_start(
                    out=xt[:, jj * CHUNK_F:(jj + nn) * CHUNK_F],
                    in_=x_ap[:, :, (plane0 + jj) * CHUNK_F:
                             (plane0 + jj + nn) * CHUNK_F])
            xts[grp] = xt
            plane0 += gn
        # wrep streams in behind x (first needed when group-0 gather starts)
        wrep = wp.tile([128, NTERM * GMAX * XPAD], F16)
        WHF = NTERM * GMAX * XPAD
        nc.sync.dma_start(out=wrep[:, 0:WHF // 2], in_=wreph_ap[:, 0:WHF // 2])
        nc.sync.dma_start(out=wrep[:, WHF // 2:WHF],
                          in_=wreph_ap[:, WHF // 2:WHF])

        # HAM warmup while x loads
        ps_w = ps1p.tile([128, 1024], F32, tag="ps1", name="ps_warm")
        for _ in range(WARMUP_MM):
            nc.tensor.matmul(ps_w[:, 0:128], ident, wts[:, 0:128],
                             start=True, stop=True)
        del ps_w

        yvalls = {}
        plane0 = 0
        for grp, gn in enumerate(GROUPS):
            planes = list(range(plane0, plane0 + gn))
            plane0 += gn
            xt = xts[grp]
            gX = gn * XPAD
            simflat = sfp.tile([16 * gn, CHUNK_F], F16, tag="simflat")

            # ---- conv chain: 2-plane rounds share each stationary ----
            for k, (fo, fn) in enumerate(BLOCKS):
                ps3 = ps3p.tile([32 * ((gn + 1) // 2), 512], F32, tag="ps3")
                for jp in range(0, gn, 2):
                    js = [jp] if jp + 1 >= gn else [jp, jp + 1]
                    ps1s, h1s = [], []
                    for j in js:
                        xv = xt[:, j * CHUNK_F:(j + 1) * CHUNK_F]
                        ps1 = ps1p.tile([128, 1024], F32, tag="ps1")
                        nc.tensor.matmul(ps1[:, 0:fn], s01[0:64, :],
                                         xv[0:64, fo:fo + fn],
                                         start=True, stop=True,
                                         tile_position=(0, 0))
                        nc.tensor.matmul(ps1[:, 512:512 + fn], s01[64:128, :],
                                         xv[64:128, fo:fo + fn],
                                         start=True, stop=True,
                                         tile_position=(64, 0))
                        ps1s.append(ps1)
                    for j, ps1 in zip(js, ps1s):
                        h1 = hp.tile([128, 1024], F16, tag="h1")
                        r1e = R1_ENG[((planes[j]) * 3 + k) % len(R1_ENG)]
                        if fn == 512:
                            if r1e == "a":
                                nc.scalar.activation(h1[:, 0:1024],
                                                     ps1[:, 0:1024], AF.Relu)
                            else:
                                nc.vector.tensor_scalar_max(
                                    h1[:, 0:1024], ps1[:, 0:1024], 0.0)
                        else:
                            for so in (0, 512):
                                if r1e == "a":
                                    nc.scalar.activation(
                                        h1[:, so:so + fn],
                                        ps1[:, so:so + fn], AF.Relu)
                                else:
                                    nc.vector.tensor_scalar_max(
                                        h1[:, so:so + fn],
                                        ps1[:, so:so + fn], 0.0)
                        h1s.append(h1)
                    # per-plane ps2 tiles (1 bank each, pool bufs=2): adjacent
                    # rounds rotate banks so stage-2 matmuls never wait on the
                    # previous round's relu2 drain
                    ps2s = []
                    for i, (j, h1) in enumerate(zip(js, h1s)):
                        ps2 = ps2p.tile([128, 512], F32, tag="ps2")
                        nc.tensor.matmul(ps2[0:64, 0:fn], ta,
                                         h1[:, 0:fn], start=True, stop=True,
                                         tile_position=(0, 0))
                        nc.tensor.matmul(ps2[64:128, 0:fn], ta,
                                         h1[:, 512:512 + fn],
                                         start=True, stop=True,
                                         tile_position=(0, 64))
                        ps2s.append(ps2)
                    h2 = h2p.tile([128, 1024], F16, tag="h2")
                    r2e = R2_ENG[((planes[jp]) * 3 + k) % len(R2_ENG)]
                    for i in range(len(js)):
                        co = 512 * i
                        if r2e == "a":
                            nc.scalar.activation(h2[:, co:co + fn],
                                                 ps2s[i][:, 0:fn], AF.Relu)
                        else:
                            nc.vector.tensor_scalar_max(
                                h2[:, co:co + fn], ps2s[i][:, 0:fn], 0.0)
                    pr = jp // 2
                    for i, j in enumerate(js):
                        nc.tensor.matmul(ps3[32 * pr:32 * pr + 32, 0:fn],
                                         l3e if i == 0 else l3o,
                                         h2[:, 512 * i:512 * i + fn],
                                         start=(i == 0), stop=(i == len(js) - 1),
                                         tile_position=(0, 32 * pr))
                se = SE_ENG[(grp * 3 + k) % len(SE_ENG)]
                if se == "a":
                    nc.scalar.copy(simflat[:, fo:fo + fn], ps3[0:16 * gn, 0:fn])
                elif se == "g":
                    nc.gpsimd.tensor_copy(simflat[:, fo:fo + fn],
                                          ps3[0:16 * gn, 0:fn])
                else:
                    nc.vector.tensor_copy(simflat[:, fo:fo + fn], ps3[0:16 * gn, 0:fn])

            # ---- store sim to padded scratch, reload 5 y-shift variants ----
            yvall = yvp.tile([128, YOFF + 5 * gX + 8], F16, tag="yvall")
            nc.gpsimd.memset(yvall[:, 0:YOFF], 0)
            nc.gpsimd.memset(yvall[:, YOFF + 5 * gX:YOFF + 5 * gX + 8], 0)
            p0 = planes[0]
            scr = scr_ts[grp]
            SCRP = (H + 8) * W          # 21760 elements per scr plane
            # per-plane DMAs: merging these into group-wide strided DMAs was
            # measured MUCH slower (one ring/engine serializes the transfers
            # and every reload then waits on all planes' store)
            for j, p in enumerate(planes):
                sf = simflat[16 * j:16 * j + 16, :]
                nc.sync.dma_start(
                    out=scr[j, 4:132, :],
                    in_=sf.rearrange("c (r x) -> c r x", x=W))
                top = sf[0:1, :].rearrange("o (r x) -> o r x", x=W)
                nc.gpsimd.dma_start(out=scr[j, 0:4, :], in_=top[:, 4:0:-1, :])
                bot = sf[15:16, :].rearrange("o (r x) -> o r x", x=W)
                nc.gpsimd.dma_start(out=scr[j, 132:136, :],
                                    in_=bot[:, 6:2:-1, :])
                dst = _ap(yvall, YOFF + j * XPAD + 4,
                          [list(yvall.ap[0]), [gX, 5], [1, W]])
                src_base = scr[j, 0:1, 0:1]
                srcp = bass.AP(tensor=src_base.tensor, offset=src_base.offset,
                               ap=[[W, 128], [2 * W, 5], [1, W]])
                rld = nc.sync if RLD_ENG == "s" else nc.gpsimd
                rld.dma_start(out=dst, in_=srcp)
            # x-edge reflect pads for all shifts/planes (gpsimd: tiny ops,
            # keeps DVE free so the next group's muls aren't gated; measured
            # better here than on the scalar queue)
            lp_d = _ap(yvall, YOFF, [list(yvall.ap[0]), [XPAD, 5 * gn], [1, 4]])
            lp_s = _ap(yvall, YOFF + 8,
                       [list(yvall.ap[0]), [XPAD, 5 * gn], [-1, 4]])
            nc.gpsimd.tensor_copy(lp_d, lp_s)
            rp_d = _ap(yvall, YOFF + 164,
                       [list(yvall.ap[0]), [XPAD, 5 * gn], [1, 4]])
            rp_s = _ap(yvall, YOFF + 162,
                       [list(yvall.ap[0]), [XPAD, 5 * gn], [-1, 4]])
            nc.gpsimd.tensor_copy(rp_d, rp_s)
            yvalls[grp] = (yvall, planes)

        # ---- pass 2: gather muls + reduction + output per group ----
        # Emitted after ALL conv code so PE-tree matmuls land behind every
        # conv matmul in the in-order Tensor queue (a PE tree for group 0
        # emitted mid-stream would stall group 1's conv).
        plane0 = 0
        for grp, gn in enumerate(GROUPS):
            yvall, planes = yvalls[grp]
            gX = gn * XPAD
            # gather muls: P[t] = wrep[t] * yvall[slot_t, shifted dx].
            # gpsimd takes the LAST terms: the DVE add tree's first level only
            # needs t0..t15, and t16 is needed last of all
            P = ptp.tile([128, NTERM * gX], F16, tag="gtmp")
            for i, (slot, dx) in enumerate(TERMS):
                srcv = _ap(yvall, YOFF + slot * gX + dx,
                           [list(yvall.ap[0]), [1, gX]])
                w_b = _ap(wrep, i * GMAX * XPAD,
                          [list(wrep.ap[0]), [1, gX]])
                dst = P[:, i * gX:(i + 1) * gX]
                eng = nc.gpsimd if i >= NTERM - MUL_GP else nc.vector
                eng.tensor_tensor(dst, w_b, srcv, OP.mult)

            if TREE_ENG[grp % len(TREE_ENG)] == "h":
                # hybrid: DVE reduces its own nv products to one slab at P[0],
                # PE identity-accumulates [gpsimd products..., slab] — only
                # (MUL_GP+1)*2 matmuls, and the DVE adds never wait on gpsimd
                nv = NTERM - MUL_GP
                rem = nv
                while rem > 1:
                    half = rem // 2
                    nc.vector.tensor_tensor(
                        P[:, 0:half * gX], P[:, 0:half * gX],
                        P[:, half * gX:2 * half * gX], OP.add)
                    if rem % 2:
                        nc.vector.tensor_tensor(
                            P[:, 0:gX], P[:, 0:gX],
                            P[:, (rem - 1) * gX:rem * gX], OP.add)
                    rem = half
                psg = ps1p.tile([128, 1024], F32, tag="ps1", name=f"psg{grp}")
                hw = gX // 2
                chain = list(range(nv, NTERM)) + [0]
                for ci, t in enumerate(chain):
                    for so, po in ((0, 0), (hw, 512)):
                        nc.tensor.matmul(
                            psg[:, po:po + hw], ident,
                            P[:, t * gX + so:t * gX + so + hw],
                            start=(ci == 0), stop=(ci == len(chain) - 1))
                of32 = ofp.tile([128, gX], F32, tag="of32")
                nc.scalar.copy(of32[:, 0:hw], psg[:, 0:hw])
                nc.scalar.copy(of32[:, hw:gX], psg[:, 512:512 + hw])
            elif TREE_ENG[grp % len(TREE_ENG)] == "p":
                # PE identity-accumulate: 17 terms x 2 halves into 2 psum banks
                psg = ps1p.tile([128, 1024], F32, tag="ps1", name=f"psg{grp}")
                hw = gX // 2
                for t in range(NTERM):
                    nc.tensor.matmul(psg[:, 0:hw], ident,
                                     P[:, t * gX:t * gX + hw],
                                     start=(t == 0), stop=(t == NTERM - 1))
                for t in range(NTERM):
                    nc.tensor.matmul(psg[:, 512:512 + hw], ident,
                                     P[:, t * gX + hw:(t + 1) * gX],
                                     start=(t == 0), stop=(t == NTERM - 1))
                of32 = ofp.tile([128, gX], F32, tag="of32")
                nc.scalar.copy(of32[:, 0:hw], psg[:, 0:hw])
                nc.scalar.copy(of32[:, hw:gX], psg[:, 512:512 + hw])
            else:
                # DVE flat add tree over 17 slices: 16 -> 8 -> 4 -> 2 -> 1, +last
                of32 = ofp.tile([128, gX], F32, tag="of32")
                nc.vector.tensor_tensor(P[:, 0:8 * gX], P[:, 0:8 * gX],
                                        P[:, 8 * gX:16 * gX], OP.add)
                nc.vector.tensor_tensor(P[:, 0:4 * gX], P[:, 0:4 * gX],
                                        P[:, 4 * gX:8 * gX], OP.add)
                nc.vector.tensor_tensor(P[:, 0:2 * gX], P[:, 0:2 * gX],
                                        P[:, 2 * gX:4 * gX], OP.add)
                nc.vector.tensor_tensor(P[:, 0:gX], P[:, 0:gX],
                                        P[:, gX:2 * gX], OP.add)
                nc.vector.tensor_tensor(of32[:, :], P[:, 0:gX],
                                        P[:, 16 * gX:17 * gX], OP.add)
            nc.scalar.dma_start(
                out=out_ap[planes[0]:planes[0] + gn].rearrange("d h x -> h d x"),
                in_=_ap(of32, 4, [list(of32.ap[0]), [XPAD, gn], [1, W]]))

    nc.compile()
    return nc


def _pack_weights(w0, bn0_scale, bn0_bias, w1, bn1_scale, bn1_bias, w_sim, b_sim):
    w0f = (w0 * bn0_scale[:, None]).astype(np.float32)
    w1f = (w1 * bn1_scale[:, None]).astype(np.float32)
    s01 = np.zeros((128, 128), np.float16)
    for half in range(2):
        for a in range(8):
            for g in range(G):
                for o in range(16):
                    s01[64 * half + 8 * a + g, 16 * a + o] = w0f[o, g]
    ta = np.zeros((128, 64), np.float16)
    for a in range(8):
        for o in range(16):
            for q in range(8):
                ta[16 * a + o, 8 * a + q] = w1f[q, o]
    l3 = np.zeros((128, 64), np.float16)
    for c in range(NCHUNK):
        l3[c * 8:c * 8 + 8, c] = w_sim[0, :]          # l3even: cols 0-15
        l3[c * 8:c * 8 + 8, 32 + 16 + c] = w_sim[0, :]  # l3odd: cols 48-63
    ident = np.eye(128, dtype=np.float16)
    return np.hstack([s01, ta, l3, ident])


def prepare(x1, offset, weight, w0, bn0_scale, bn0_bias, w1, bn1_scale, bn1_bias,
            w_sim, b_sim):
    x1 = np.asarray(x1); offset = np.asarray(offset); weight = np.asarray(weight)
    w0 = np.asarray(w0); bn0_scale = np.asarray(bn0_scale)
    bn0_bias = np.asarray(bn0_bias); w1 = np.asarray(w1)
    bn1_scale = np.asarray(bn1_scale); bn1_bias = np.asarray(bn1_bias)
    w_sim = np.asarray(w_sim); b_sim = np.asarray(b_sim)

    wts = _pack_weights(w0, bn0_scale, bn0_bias, w1, bn1_scale, bn1_bias,
                        w_sim, b_sim)
    zb0 = bool(np.all(bn0_bias == 0))
    zb1 = bool(np.all(bn1_bias == 0))
    zbsim = bool(np.all(b_sim == 0))
    nc = _build(zb0, zb1, zbsim)

    # host-prefolded wrep: [r, t*GMAX*XPAD + j*XPAD + xp], zero x-pads
    ws_list = list(range(9)) + [9 + s for s in range(9) if s != 4]
    wreps = []
    for b in range(B):
        wall = (offset[b] * (0.5 * weight[b, 0])[None]).astype(np.float32)
        wall[4] += wall[13]
        wsel = np.zeros((NTERM, H, XPAD), np.float16)
        wsel[:, :, 4:4 + W] = wall[ws_list]
        wr = wsel.transpose(1, 0, 2)[:, :, None, :]          # [H, t, 1, XPAD]
        wr = np.broadcast_to(wr, (H, NTERM, GMAX, XPAD))
        wreps.append(np.ascontiguousarray(wr).reshape(
            128, NTERM * GMAX * XPAD))

    in_maps = []
    for core in range(8):
        b, kd = divmod(core, 4)
        xs = x1[b, :, kd * DG:(kd + 1) * DG].astype(np.float16)
        xs = xs.reshape(G, DG, NCHUNK, RPC, W)
        xs = np.ascontiguousarray(xs.transpose(2, 0, 1, 3, 4)).reshape(
            NCHUNK, G, DG * CHUNK_F)
        in_maps.append({"x": xs, "wts": wts, "wreph": wreps[b]})
    return nc, in_maps


def kernel(x1, offset, weight, w0, bn0_scale, bn0_bias, w1, bn1_scale, bn1_bias,
           w_sim, b_sim):
    nc, in_maps = prepare(x1, offset, weight, w0, bn0_scale, bn0_bias, w1,
                          bn1_scale, bn1_bias, w_sim, b_sim)
    res = run_bass_kernel_spmd(nc, in_maps, list(range(8)))
    out = np.empty((B, D, H, W), np.float32)
    for core in range(8):
        b, kd = divmod(core, 4)
        out[b, kd * DG:(kd + 1) * DG] = res.results[core]["out"]
    return out

